# revision 18
# baseline (speedup 1.0000x reference)
"""Trainium2 Bass kernel: per-channel broadcast multiply (ChannelMultiplier).

out[n, c, h, w] = x[n, c, h, w] * multiplier[c]

x: (32, 256, 56, 56) f32, multiplier: (256,) f32.

Sharding: data-parallel over the batch dim N across 8 NeuronCores
(4 batches per core); the 1 KB multiplier is replicated to every core.

Per-core layout: the local shard (4, 256, 56, 56) is viewed row-major as
(1024, 3136); row r = n*256 + c is one (n, c) image plane of 3136
contiguous floats.  Grouping rows as (n, h, p) with h = channel half
(C = 256 = 2*128) puts a FIXED channel on each SBUF partition, so a
whole [128, n-group, 3136] tile is scaled by a single per-partition
vector (a half of `multiplier`) in ONE vector-engine tensor_scalar_mul.

This build runs under a walrus config (DynamicDMA disabled) where every
instruction — compute and pseudo-DMA alike — has exactly ONE semaphore
wait slot, and the 8 HWDGE completion lanes (DMAHW0-7) are serialized
per lane (a lane's 2nd DMA waits on its 1st).  The schedule is shaped so
no instruction ever needs two waits:
  * exactly 8 HWDGE DMAs (4 loads + 4 stores) -> every lane is used once;
  * the tiny scale DMA goes through SWDGE (gpsimd), a separate lane pool;
  * 4 distinct SBUF slots, one per tile: loads have no WAR waits;
  * each DVE multiply waits only on its own load's DMA lane and fully
    shadows the load's write, so each store waits only on the DVE sem;
  * the per-partition scalar operand of TensorScalar is read in the
    engine's setup phase (a pointer-read hazard needing one sem wait at
    the first consumer), so a warm-up op takes that wait once.
"""

import numpy as np

import concourse.bacc as bacc
import concourse.bass as bass
import concourse.mybir as mybir
import concourse.tile as tile_mod
from concourse.bass_utils import run_bass_kernel_spmd
from concourse.tile import TileContext

N, C, H, W = 32, 256, 56, 56
N_CORES = 8
NL = N // N_CORES  # batches per core
P = 128  # SBUF partitions
F = H * W  # 3136 contiguous floats per (n, c) row
ROWS = NL * C  # 1024 rows per core
HALVES = C // P  # 2 channel halves
FSPLIT = 2  # f-dim slices per (batch, half)
FS = F // FSPLIT  # 1568 floats per slice
NTILES = NL * HALVES * FSPLIT  # 16 tiles of [128, FS] per core

_NC_CACHE: list = [None]


def _build() -> bass.Bass:
    # Bacc (not raw Bass): its finalize() runs generate_event_semaphores,
    # which splits multi-wait sync_info into InstEventSemaphore chains —
    # engine ISA words only carry one semaphore wait each.
    nc = bacc.Bacc()
    x = nc.declare_dram_parameter("x", [ROWS, F], mybir.dt.float32, isOutput=False)
    mult = nc.declare_dram_parameter("multiplier", [C], mybir.dt.float32, isOutput=False)
    y = nc.declare_dram_parameter("y", [ROWS, F], mybir.dt.float32, isOutput=True)

    # [n, h, s, p, f]: tile (n, h, s) holds f-slice s of channels
    # h*128..h*128+127 of batch n, one channel per partition.
    xv = x.rearrange("(n h p) (s f) -> n h s p f", h=HALVES, p=P, s=FSPLIT)
    yv = y.rearrange("(n h p) (s f) -> n h s p f", h=HALVES, p=P, s=FSPLIT)
    # [p, h]: column h holds multiplier[h*128 + p].
    mv = mult.rearrange("(h p) -> p h", h=HALVES)

    with TileContext(nc) as tc:
        with (
            tc.tile_pool(name="scale", bufs=1) as spool,
            tc.tile_pool(name="data", bufs=NTILES) as pool,
        ):
            # Scale staging: SWDGE DMA -> sc, DVE copy -> sc2 (takes the
            # DMA wait), warm-up TensorScalar consumes sc2's pointer
            # (takes the same-engine pointer-read hazard wait).
            sc = spool.tile([P, HALVES], mybir.dt.float32, tag="sc")
            nc.gpsimd.dma_start(out=sc[:, :], in_=mv)
            sc2 = spool.tile([P, HALVES], mybir.dt.float32, tag="sc2")
            nc.vector.tensor_copy(out=sc2[:, :], in_=sc[:, :])
            scr = spool.tile([P, HALVES], mybir.dt.float32, tag="scr")
            warm = nc.vector.tensor_scalar_mul(scr[:, :], sc2[:, :], sc2[:, 0:1])

            # All loads first: they dispatch back-to-back from SP with no
            # waits, so DMA bandwidth is busy from t=0.
            def tile_idx(t):
                n, r = divmod(t, HALVES * FSPLIT)
                h, s = divmod(r, FSPLIT)
                return n, h, s

            tiles = []
            for t in range(NTILES):
                n, h, s = tile_idx(t)
                tile = pool.tile([P, FS], mybir.dt.float32, tag="data", bufs=NTILES)
                nc.sync.dma_start(out=tile[:, :], in_=xv[n, h, s])
                tiles.append(tile)

            for t in range(NTILES):
                n, h, s = tile_idx(t)
                tile = tiles[t]
                mul = nc.vector.tensor_scalar_mul(
                    tile[:, :], tile[:, :], sc2[:, h : h + 1]
                )
                # Keep the warm-up ahead of every scalar-pointer consumer
                # in the DVE stream (ordering only, no semaphore).
                tile_mod.add_dep_helper(
                    mul.ins, warm.ins, sync=False, reason="scale ptr hazard warm-up"
                )
                # Store from the ACT engine's HWDGE: its single wait slot
                # takes the DVE semaphore.
                nc.scalar.dma_start(out=yv[n, h, s], in_=tile[:, :])
    nc.finalize()
    return nc


def _get_nc() -> bass.Bass:
    if _NC_CACHE[0] is None:
        _NC_CACHE[0] = _build()
    return _NC_CACHE[0]


def kernel(x: np.ndarray, multiplier: np.ndarray) -> np.ndarray:
    x = np.ascontiguousarray(x, dtype=np.float32)
    multiplier = np.ascontiguousarray(multiplier, dtype=np.float32)
    assert x.shape == (N, C, H, W), x.shape
    assert multiplier.shape == (C,), multiplier.shape

    xr = x.reshape(N_CORES, ROWS, F)
    in_maps = [{"x": xr[i], "multiplier": multiplier} for i in range(N_CORES)]
    res = run_bass_kernel_spmd(_get_nc(), in_maps, list(range(N_CORES)))
    out = np.concatenate(
        [r["y"].reshape(NL, C, H, W) for r in res.results], axis=0
    )
    return out
